# revision 8
# baseline (speedup 1.0000x reference)
"""GPT-2 style multi-head attention on 8 Trainium2 cores (Bass/Tile), v3.

Problem: B=2, T=2048, C=1024, H=16 heads, D=64, fp32 in/out.

Sharding (hardcoded): 8-way head-parallel. Core c computes heads
[2c, 2c+2) for BOTH batches (8 blocks of 512 tokens each, bb = b*4+tb).
After attention, ONE AllToAll per rep reshards y^T from head-sharded to
t-block-sharded: rank d ends up with the full [1024, 512] y^T for block
d and computes the full-width output projection locally (full c_proj_w,
full bias). Core d returns out rows [512*tb_d, 512*tb_d+512) of batch
b_d.

v3 vs v2 (which AllGathered y per q-block, 8 collectives/rep): the
collective queue was the bottleneck — each collective costs ~15us launch
overhead + transfer, serialized. One 1MB AllToAll (~40us) hides under
~125us of PE work.

Pipeline structure:
  - host ships x^T (c-major) in bf16: no PE transposes.
  - all weights/activations bf16 (PSUM accum fp32).
  - wide exp: one ACT instruction covers both heads' scores; the causal
    mask multiply only touches the 128-wide diagonal k-tile.
  - V bias folded into the normalize (yn = y*rec + bv as a per-partition
    scalar add) instead of a PE bias matmul.
  - software pipelining: stage1(bb+1) and proj(rep-1) matmul chunks are
    dispensed as fillers between attention(bb) chain iterations; the
    NEXT rep's tiles + weight loads + stage1(block 0) are created at
    block 7 so the in-order PE never drains at the rep boundary.
  - DMA queues by dependency class: SP = input streams + compute-near
    writes; gpsimd(SWDGE) = weights, collective, and A2A-dependent
    proj loads.
  - persistent tensors double-buffered across timing reps (tag rep%2).
"""

import numpy as np

import concourse.bass as bass
import concourse.mybir as mybir
import concourse.tile as tile
from concourse import bacc

P = 128
B, T_FULL, C, H, D = 2, 2048, 1024, 16, 64
F32 = mybir.dt.float32
F32R = mybir.dt.float32r
BF16 = mybir.dt.bfloat16
EXP = mybir.ActivationFunctionType.Exp
COPY = mybir.ActivationFunctionType.Copy
VW = 128  # per-head V stride: 64 V cols + 64 ones cols
HL = 2   # heads per core
NB = 8   # (batch, t-block) blocks
QKCH = 2  # qkT m-chunks: 0=Q, 1=K


class Cfg:
    def __init__(self, n_cores, group_size, T, fake_collective=False,
                 repeat=1):
        self.fake_collective = fake_collective
        self.repeat = repeat
        self.n_cores = n_cores
        self.T = T
        self.HL = HL
        self.CC = C // P                   # contraction chunks (8)
        self.TB = T // 512                 # t-blocks per batch (4)
        self.KT = T // P                   # k tiles per batch (16)
        if n_cores == 8:
            self.replica_groups = [[0, 1, 2, 3, 4, 5, 6, 7]]
        elif n_cores == 1:
            self.replica_groups = [[0]]
        else:
            raise ValueError(n_cores)


CFG_FULL = Cfg(8, 4, T_FULL)

# block at which the previous rep's proj fillers (dependent on the
# previous rep's AllToAll) are dispensed. Block 7 has no stage1 fillers
# of its own (there is no block 8), and the A2A is long done by then.
PROJ_BLOCK = 7


class _RepCtx:
    """Per-rep tiles + stage1 closures. Created one rep AHEAD (at block
    7 of the previous rep) so the next rep's weight DMAs are queued
    before the previous rep's AllToAll and its stage1(0) can fill the
    previous rep's last attention block."""

    def __init__(self, tc, ins, cfg, rep, pools, state):
        nc = tc.nc
        self.nc = nc
        self.cfg = cfg
        self.rep = rep
        self.pools = pools
        db = rep % 2
        persist = pools["persist"]
        dram = pools["dram"]
        s1 = pools["s1"]

        self.qkT = persist.tile([P, QKCH, NB * 512], BF16, tag=f"qkT{db}",
                                name=f"qkT_{rep}")
        self.vsb = persist.tile([P, 2 * cfg.KT, HL * VW], BF16,
                                tag=f"vsb{db}", name=f"vsb_{rep}")
        self.mask_sb = persist.tile([P, 4, 512], BF16, tag=f"mask{db}",
                                    name=f"mask_{rep}")
        self.wqk_sb = persist.tile([P, cfg.CC, QKCH * P], BF16,
                                   tag=f"wqk{db}", name=f"wqk_{rep}")
        self.wv_sb = persist.tile([P, cfg.CC, HL * D], BF16,
                                  tag=f"wv{db}", name=f"wv_{rep}")
        self.wp_sb = persist.tile([P, cfg.CC, C], BF16, tag=f"wp{db}",
                                  name=f"wp_{rep}")
        self.bqk_sb = persist.tile([P, QKCH], F32, tag=f"bqk{db}",
                                   name=f"bqk_{rep}")
        self.bv_sb = persist.tile([D, HL], F32, tag=f"bv{db}",
                                  name=f"bv_{rep}")
        self.bp_sb = persist.tile([1, C], BF16, tag=f"bp{db}",
                                  name=f"bp_{rep}")
        self.a2a_in = dram.tile([NB * P, 512], BF16, tag=f"a2ain{db}",
                                name=f"a2ain_{rep}")
        self.a2a_out = dram.tile([NB * P, 512], BF16, tag=f"a2aout{db}",
                                 name=f"a2aout_{rep}")
        self.xT_r = ins["xT"].rearrange("p (bb c u) -> p bb c u",
                                        bb=NB, c=cfg.CC)

        # weights on the gpsimd (SWDGE) queue, emitted at rep start so
        # they dispatch AFTER the previous rep's AllToAll (emitting them
        # earlier would delay the A2A dispatch on this serial queue).
        nc.sync.dma_start(self.mask_sb[:], ins["masks"])
        nc.gpsimd.dma_start(
            self.wqk_sb[:], ins["wqk"].rearrange("p (c m) -> p c m",
                                                 c=cfg.CC))
        nc.gpsimd.dma_start(
            self.wv_sb[:], ins["wv"].rearrange("p (c m) -> p c m",
                                               c=cfg.CC))
        nc.gpsimd.dma_start(self.bqk_sb[:], ins["bqk"])
        nc.gpsimd.dma_start(self.bv_sb[:], ins["bv"])
        nc.gpsimd.dma_start(
            self.wp_sb[:], ins["wp"].rearrange("p (c n) -> p c n",
                                               c=cfg.CC))
        nc.gpsimd.dma_start(self.bp_sb[:], ins["bp"])

        # constants: ones_row is a singleton written once in rep 0; the
        # vsb ones-columns (cols 64:128 of each head, which replicate
        # the softmax denominator into PSUM partitions 64:128 during the
        # AV matmul) are written once per double-buffer slot — v_half
        # only ever writes cols 0:64.
        self.vsb_h = self.vsb.rearrange("p k (h w) -> p k h w", w=VW)
        if rep == 0:
            state["ones_row"] = persist.tile([1, P], BF16, tag="ones_row",
                                             name="ones_row0")
            nc.vector.memset(state["ones_row"][:], 1.0)
        self.ones_row = state["ones_row"]
        if rep < 2:
            nc.vector.memset(self.vsb_h[:, :, :, 64:VW], 1.0)

    def xt_load(self, bb):
        xt = self.pools["s1"].tile([P, self.cfg.CC, 512], BF16, tag="xt",
                                   bufs=3, name=f"xt{bb}_{self.rep}")
        self.nc.sync.dma_start(xt[:], self.xT_r[:, bb, :, :])
        return xt

    def qk_quarter(self, bb, m, xt, quarter, st):
        nc = self.nc
        CC = self.cfg.CC
        if quarter == 0:
            st["acc"] = self.pools["ps_acc"].tile(
                [P, 512], F32, tag="acc", name=f"qkacc{bb}_{m}_{self.rep}")
        acc = st["acc"]
        for cc in range(quarter * 2, quarter * 2 + 2):
            nc.tensor.matmul(
                acc[:],
                self.wqk_sb[:, cc, m * P:(m + 1) * P],
                xt[:, cc, :],
                start=(cc == 0),
                stop=(cc == CC - 1),
            )
        if quarter == 3:
            nc.vector.tensor_scalar_add(
                self.qkT[:, m, bb * 512:(bb + 1) * 512], acc[:],
                self.bqk_sb[:, m:m + 1],
            )

    def v_half(self, bb, ts, xt, half, st):
        nc = self.nc
        kt_g = bb * 4 + ts
        if half == 0:
            st["vp"] = self.pools["ps_acc"].tile(
                [P, 512], F32, tag="acc", name=f"vacc{bb}_{ts}_{self.rep}")
        vp = st["vp"]
        for cc in range(half * 4, half * 4 + 4):
            nc.tensor.matmul(
                vp[:, 0:HL * D],
                xt[:, cc, ts * P:(ts + 1) * P],
                self.wv_sb[:, cc, :],
                start=(cc == 0),
                stop=(cc == self.cfg.CC - 1),
            )
        if half == 1:
            nc.vector.tensor_copy(
                self.vsb_h[:, kt_g, :, 0:64],
                vp[:, 0:HL * D].rearrange("p (h d) -> p h d", d=D),
            )

    def stage1_fillers(self, bb):
        xt = self.xt_load(bb)
        fs = []
        for m in range(QKCH):
            st = {}
            for q in range(4):
                fs.append(
                    lambda m=m, q=q, st=st: self.qk_quarter(bb, m, xt, q, st))
        for ts in range(4):
            vst = {}
            fs.append(lambda ts=ts, st=vst: self.v_half(bb, ts, xt, 0, st))
            fs.append(lambda ts=ts, st=vst: self.v_half(bb, ts, xt, 1, st))
        return fs


def emit(tc, outs, ins, cfg):
    from contextlib import ExitStack
    with ExitStack() as _stk:
        pools = dict(
            persist=_stk.enter_context(tc.tile_pool(name="persist", bufs=1)),
            s1=_stk.enter_context(tc.tile_pool(name="s1", bufs=2)),
            s2=_stk.enter_context(tc.tile_pool(name="s2", bufs=4)),
            s3=_stk.enter_context(tc.tile_pool(name="s3", bufs=2)),
            dram=_stk.enter_context(
                tc.tile_pool(name="dram", bufs=1, space="DRAM")),
            ps_acc=_stk.enter_context(tc.tile_pool(
                name="ps_acc", bufs=3, space="PSUM")),
            ps_s=_stk.enter_context(tc.tile_pool(
                name="ps_s", bufs=3, space="PSUM")),
            ps_y=_stk.enter_context(tc.tile_pool(
                name="ps_y", bufs=2, space="PSUM")),
        )
        carry = []
        state = {}
        for rep in range(cfg.repeat):
            carry = _emit_once(tc, outs["out"], ins, cfg, rep, pools, carry,
                               state)
        for f in carry:
            f()


def _emit_once(tc, out, ins, cfg, rep, pools, carry, state):
    nc = tc.nc
    CC = cfg.CC
    ctx = _RepCtx(tc, ins, cfg, rep, pools, state)
    for f in ctx.stage1_fillers(0):
        f()

    s2 = pools["s2"]
    s3 = pools["s3"]
    ps_acc = pools["ps_acc"]
    ps_s = pools["ps_s"]
    ps_y = pools["ps_y"]
    qkT, vsb, mask_sb = ctx.qkT, ctx.vsb, ctx.mask_sb
    ones_row = ctx.ones_row

    # ---- proj chunks (for THIS rep; dispensed during the NEXT rep) ----
    a2a_out_r = ctx.a2a_out.rearrange("(c p) t -> p c t", p=P)

    def proj_load(st):
        # full y^T [1024, 512] for my block: 1KB runs per (p, chunk)
        ag = s3.tile([P, CC, 512], BF16, tag="ag")
        nc.gpsimd.dma_start(ag[:], a2a_out_r[:])
        st["ag"] = ag

    def proj_half(tsub, hh, half, st, pst):
        col = slice(tsub * P, (tsub + 1) * P)
        if half == 0:
            pst["op"] = ps_acc.tile([P, 512], F32, tag="acc",
                                    name=f"pacc{tsub}_{hh}_{rep}")
        op = pst["op"]
        for cc in range(half * 4, half * 4 + 4):
            nc.tensor.matmul(
                op[:], st["ag"][:, cc, col],
                ctx.wp_sb[:, cc, hh * 512:(hh + 1) * 512],
                start=(cc == 0), stop=False,
            )
        if half == 1:
            nc.tensor.matmul(
                op[:], ones_row[:1, :],
                ctx.bp_sb[:1, hh * 512:(hh + 1) * 512],
                start=False, stop=True,
            )
            o_sb = s3.tile([P, 512], F32, tag="osb")
            nc.scalar.activation(o_sb[:], op[:], COPY)
            nc.sync.dma_start(
                out[tsub * P:(tsub + 1) * P, hh * 512:(hh + 1) * 512],
                o_sb[:])

    def proj_fillers():
        st = {}
        fs = [lambda st=st: proj_load(st)]
        for tsub in range(4):
            for hh in range(2):
                pst = {}
                fs.append(lambda tsub=tsub, hh=hh, pst=pst, st=st:
                          proj_half(tsub, hh, 0, st, pst))
                fs.append(lambda tsub=tsub, hh=hh, pst=pst, st=st:
                          proj_half(tsub, hh, 1, st, pst))
        return fs

    # ---- attention with filler dispensing ----
    def attention(bb, early, late, pre):
        # early: fillers safe to run from the start (stage1 of bb+1)
        # late: fillers needing the previous rep's AllToAll (proj)
        # pre: deferred work (previous block's normalize) that MUST be
        #      emitted before this block's first AV (ps_y WAR).
        b, qb = divmod(bb, 4)
        nkt = 4 * qb + 4
        kt_order = list(range(4 * qb, nkt)) + list(range(0, 4 * qb))
        fillers = list(early)
        late = list(late)
        ci = 0  # chain iterations done

        def dispense(n):
            for _ in range(n):
                if fillers:
                    fillers.pop(0)()

        hs = (0, 1)
        pbs = [slice(h * 64, h * 64 + 64) for h in hs]
        ys = [ps_y.tile([P, 512], F32, tag="y",
                        name=f"y{bb}_{h}_{rep}") for h in hs]
        for ki, kt in enumerate(kt_order):
            if ki == 0:
                fillers.extend(late)
                late = []
            j = kt - 4 * qb
            lo = 128 * j if j > 0 else 0
            e = s2.tile([P, 2, 512], BF16, tag="e", bufs=6,
                        name=f"e{bb}_{kt}_{rep}")
            ss = []
            for i in range(2):
                s = ps_s.tile([P, 512], F32, tag="s",
                              name=f"s{bb}_{kt}_{i}_{rep}")
                ss.append(s)
                nc.tensor.matmul(
                    s[:, lo:],
                    qkT[pbs[i], 1,
                        b * cfg.T + kt * P:b * cfg.T + (kt + 1) * P],
                    qkT[pbs[i], 0, bb * 512 + lo:(bb + 1) * 512],
                    start=True, stop=True,
                )
                # per-head exp: half the chain latency of a wide one
                nc.scalar.activation(
                    e[:, i, lo:], s[:, lo:], EXP, scale=0.125)
                if j >= 0:
                    # only the diagonal 128-wide k-tile needs masking;
                    # all farther query columns are fully valid.
                    nc.vector.tensor_mul(
                        e[:, i, lo:lo + P], e[:, i, lo:lo + P],
                        mask_sb[:, j, lo:lo + P])
            if ki == 0:
                for f in pre:
                    f()
                pre = []
            ci += 1
            # dispense fillers between QK and AV: the filler matmuls
            # hide the exp->mask latency on in-order PE.
            rem_work = len(fillers) + len(late)
            rem_iter = nkt - ci
            if rem_iter > 0 and rem_work > 0:
                per = (rem_work + rem_iter - 1) // rem_iter
                dispense(min(per, 2))
            for i in range(2):
                nc.tensor.matmul(
                    ys[i][:, lo:],
                    vsb[:, b * cfg.KT + kt, hs[i] * VW:(hs[i] + 1) * VW],
                    e[:, i, lo:],
                    start=(ki == 0), stop=(ki == nkt - 1),
                )
        for f in pre:
            f()
        fillers.extend(late)
        dispense(len(fillers))
        return [lambda i=i: _normalize(bb, hs[i], ys[i]) for i in (0, 1)]

    def _normalize(bb, h, y):
        # the AV ones-columns replicated the denominator into PSUM
        # partitions 64:128; reciprocal straight into partitions 0:64.
        rec = s2.tile([64, 512], F32R, tag="rec", bufs=3,
                      name=f"rec{bb}_{h}_{rep}")
        with nc.allow_low_precision(
            reason="reciprocal of softmax denominators; ~1e-6"
            " relative is plenty"
        ):
            nc.vector.reciprocal(rec[:], y[64:128, :])
        ynf = s2.tile([64, 512], F32, tag="ynf", bufs=3,
                      name=f"ynf{bb}_{h}_{rep}")
        nc.vector.tensor_mul(ynf[:], y[0:64, :], rec[:])
        yn = s2.tile([64, 512], BF16, tag="yn", bufs=3,
                     name=f"yn{bb}_{h}_{rep}")
        # V bias folded here: y/denom + bv (per-partition scalar)
        nc.vector.tensor_scalar_add(yn[:], ynf[:], ctx.bv_sb[:, h:h + 1])
        nc.sync.dma_start(
            ctx.a2a_in[bb * P + h * D:bb * P + (h + 1) * D, :], yn[:])

    def all_to_all():
        if cfg.fake_collective:
            nc.gpsimd.dma_start(ctx.a2a_out[:], ctx.a2a_in[:])
            return
        nc.gpsimd.collective_compute(
            "AllToAll", mybir.AluOpType.bypass,
            replica_groups=cfg.replica_groups,
            ins=[ctx.a2a_in[:].opt()],
            outs=[ctx.a2a_out[:].opt()],
        )

    # ---- fused pipeline ----
    pre = []
    for bb in range(NB):
        early = ctx.stage1_fillers(bb + 1) if bb + 1 < NB else []
        late = []
        if bb == PROJ_BLOCK - 1 and carry:
            early.append(carry.pop(0))  # the a2a_out load leads by a block
        if bb == PROJ_BLOCK:
            late = list(carry)
            carry = []
        pre = attention(bb, early, late, pre)
    # block 7's normalize must precede the A2A emission
    for f in pre:
        f()
    # leftover carry (repeat=1 case): flush before the collective
    for f in carry:
        f()
    all_to_all()
    # defer this rep's proj (A2A-dependent) into the next rep
    return proj_fillers()


SHAPES = {
    "xT": ((P, NB * (C // P) * 512), BF16),
    "wqk": ((P, (C // P) * QKCH * P), BF16),
    "wv": ((P, (C // P) * HL * D), BF16),
    "bqk": ((P, QKCH), F32),
    "bv": ((D, HL), F32),
    "wp": ((P, (C // P) * C), BF16),
    "bp": ((1, C), BF16),
    "masks": ((P, 4, 512), BF16),
}


def build(cfg, num_devices=None):
    nc = bacc.Bacc("TRN2", target_bir_lowering=False, debug=False,
                   num_devices=num_devices or cfg.n_cores)
    ins = {}
    for name, (shape, dt) in SHAPES.items():
        ins[name] = nc.dram_tensor(
            name, list(shape), dt, kind="ExternalInput").ap()
    outs = {"out": nc.dram_tensor(
        "out", [512, C], F32, kind="ExternalOutput").ap()}
    with tile.TileContext(nc) as tc:
        emit(tc, outs, ins, cfg)
    nc.compile()
    return nc


def make_core_inputs(x_full, c_attn_w, c_attn_b, c_proj_w, c_proj_b, cfg,
                     core):
    import ml_dtypes
    bf = ml_dtypes.bfloat16
    T = cfg.T
    hs = slice(core * HL * D, (core + 1) * HL * D)
    wq = c_attn_w[:, 0 * C:1 * C][:, hs]
    wk = c_attn_w[:, 1 * C:2 * C][:, hs]
    wv = c_attn_w[:, 2 * C:3 * C][:, hs]
    bq = c_attn_b[0 * C:1 * C][hs]
    bk = c_attn_b[1 * C:2 * C][hs]
    bv = c_attn_b[2 * C:3 * C][hs]

    pp = np.arange(P)[:, None, None]
    jj = np.arange(4)[None, :, None]
    qq = np.arange(512)[None, None, :]
    masks = (qq >= pp + 128 * jj)

    def chunkp(w):
        # [C, m] -> [P, CC*m]: row p holds chunk-major contiguous runs,
        # so every SBUF load is one long run per partition.
        m = w.shape[1]
        return np.ascontiguousarray(
            w.reshape(C // P, P, m).transpose(1, 0, 2).reshape(P, -1)
            .astype(bf))

    # x^T for BOTH batches, block-major (bb = b*TB + tb):
    # [P, bb, cc, 512]
    xbs = []
    for b in range(B):
        xt = x_full[b, :T].T  # [C, T]
        xbs.append(xt.reshape(C // P, P, T // 512, 512)
                   .transpose(1, 2, 0, 3))  # [P, tb, cc, 512]
    xT2 = np.concatenate(xbs, axis=1).reshape(P, -1)

    return {
        "xT": np.ascontiguousarray(xT2.astype(bf)),
        "wqk": chunkp(np.concatenate([wq, wk], axis=1)),
        "wv": chunkp(wv),
        "bqk": np.ascontiguousarray(
            np.concatenate([bq, bk]).reshape(QKCH, P).T, np.float32),
        "bv": np.ascontiguousarray(
            bv.reshape(HL, D).T, np.float32),
        "wp": chunkp(c_proj_w),
        "bp": np.ascontiguousarray(c_proj_b[None, :].astype(bf)),
        "masks": masks.astype(bf),
    }


_CACHE = {}


def kernel(**inputs):
    from concourse.bass_utils import run_bass_kernel_spmd

    cfg = CFG_FULL
    x = np.asarray(inputs["x"], np.float32)
    c_attn_w = np.asarray(inputs["c_attn_w"], np.float32)
    c_attn_b = np.asarray(inputs["c_attn_b"], np.float32)
    c_proj_w = np.asarray(inputs["c_proj_w"], np.float32)
    c_proj_b = np.asarray(inputs["c_proj_b"], np.float32)

    if "nc" not in _CACHE:
        _CACHE["nc"] = build(cfg)
    nc = _CACHE["nc"]
    in_maps = [
        make_core_inputs(x, c_attn_w, c_attn_b, c_proj_w, c_proj_b, cfg,
                         core)
        for core in range(cfg.n_cores)
    ]
    res = run_bass_kernel_spmd(nc, in_maps, core_ids=list(range(cfg.n_cores)))
    out = np.empty((B, T_FULL, C), np.float32)
    for core in range(cfg.n_cores):
        b, tb = divmod(core, cfg.TB)
        out[b, tb * 512:(tb + 1) * 512, :] = res.results[core]["out"]
    return out


# revision 10
# speedup vs baseline: 1.0586x; 1.0586x over previous
"""GPT-2 style multi-head attention on 8 Trainium2 cores (Bass/Tile), v3.

Problem: B=2, T=2048, C=1024, H=16 heads, D=64, fp32 in/out.

Sharding (hardcoded): 8-way head-parallel. Core c computes heads
[2c, 2c+2) for BOTH batches (8 blocks of 512 tokens each, bb = b*4+tb).
After attention, ONE AllToAll per rep reshards y^T from head-sharded to
t-block-sharded: rank d ends up with the full [1024, 512] y^T for block
d and computes the full-width output projection locally (full c_proj_w,
full bias). Core d returns out rows [512*tb_d, 512*tb_d+512) of batch
b_d.

v3 vs v2 (which AllGathered y per q-block, 8 collectives/rep): the
collective queue was the bottleneck — each collective costs ~15us launch
overhead + transfer, serialized. One 1MB AllToAll (~40us) hides under
~125us of PE work.

Pipeline structure:
  - host ships x^T (c-major) in bf16: no PE transposes.
  - all weights/activations bf16 (PSUM accum fp32).
  - wide exp: one ACT instruction covers both heads' scores; the causal
    mask multiply only touches the 128-wide diagonal k-tile.
  - V bias folded into the normalize (yn = y*rec + bv as a per-partition
    scalar add) instead of a PE bias matmul.
  - software pipelining: stage1(bb+1) and proj(rep-1) matmul chunks are
    dispensed as fillers between attention(bb) chain iterations; the
    NEXT rep's tiles + weight loads + stage1(block 0) are created at
    block 7 so the in-order PE never drains at the rep boundary.
  - DMA queues by dependency class: SP = input streams + compute-near
    writes; gpsimd(SWDGE) = weights, collective, and A2A-dependent
    proj loads.
  - persistent tensors double-buffered across timing reps (tag rep%2).
"""

import numpy as np

import concourse.bass as bass
import concourse.mybir as mybir
import concourse.tile as tile
from concourse import bacc

P = 128
B, T_FULL, C, H, D = 2, 2048, 1024, 16, 64
F32 = mybir.dt.float32
F32R = mybir.dt.float32r
BF16 = mybir.dt.bfloat16
EXP = mybir.ActivationFunctionType.Exp
COPY = mybir.ActivationFunctionType.Copy
VW = 128  # per-head V stride: 64 V cols + 64 ones cols
HL = 2   # heads per core
NB = 8   # (batch, t-block) blocks
QKCH = 2  # qkT m-chunks: 0=Q, 1=K


class Cfg:
    def __init__(self, n_cores, group_size, T, fake_collective=False,
                 repeat=1):
        self.fake_collective = fake_collective
        self.repeat = repeat
        self.n_cores = n_cores
        self.T = T
        self.HL = HL
        self.CC = C // P                   # contraction chunks (8)
        self.TB = T // 512                 # t-blocks per batch (4)
        self.KT = T // P                   # k tiles per batch (16)
        if n_cores == 8:
            self.replica_groups = [[0, 1, 2, 3, 4, 5, 6, 7]]
        elif n_cores == 1:
            self.replica_groups = [[0]]
        else:
            raise ValueError(n_cores)


CFG_FULL = Cfg(8, 4, T_FULL)

# block at which the previous rep's proj fillers (dependent on the
# previous rep's AllToAll) are dispensed. Block 7 has no stage1 fillers
# of its own (there is no block 8), and the A2A is long done by then.
PROJ_BLOCK = 7


class _RepCtx:
    """Per-rep tiles + stage1 closures. Created one rep AHEAD (at block
    7 of the previous rep) so the next rep's weight DMAs are queued
    before the previous rep's AllToAll and its stage1(0) can fill the
    previous rep's last attention block."""

    def __init__(self, tc, ins, cfg, rep, pools, state):
        nc = tc.nc
        self.nc = nc
        self.cfg = cfg
        self.rep = rep
        self.pools = pools
        db = rep % 2
        persist = pools["persist"]
        dram = pools["dram"]
        s1 = pools["s1"]

        self.qkT = persist.tile([P, QKCH, NB * 512], BF16, tag=f"qkT{db}",
                                name=f"qkT_{rep}")
        self.vsb = persist.tile([P, 2 * cfg.KT, HL * VW], BF16,
                                tag=f"vsb{db}", name=f"vsb_{rep}")
        self.mask_sb = persist.tile([P, 4, 512], BF16, tag=f"mask{db}",
                                    name=f"mask_{rep}")
        self.wqk_sb = persist.tile([P, cfg.CC, QKCH * P], BF16,
                                   tag=f"wqk{db}", name=f"wqk_{rep}")
        self.wv_sb = persist.tile([P, cfg.CC, HL * D], BF16,
                                  tag=f"wv{db}", name=f"wv_{rep}")
        self.wp_sb = persist.tile([P, cfg.CC, C], BF16, tag=f"wp{db}",
                                  name=f"wp_{rep}")
        self.bqk_sb = persist.tile([P, QKCH], F32, tag=f"bqk{db}",
                                   name=f"bqk_{rep}")
        self.bv_sb = persist.tile([D, HL], F32, tag=f"bv{db}",
                                  name=f"bv_{rep}")
        self.bp_sb = persist.tile([1, C], BF16, tag=f"bp{db}",
                                  name=f"bp_{rep}")
        self.a2a_in = dram.tile([NB * P, 512], BF16, tag=f"a2ain{db}",
                                name=f"a2ain_{rep}")
        self.a2a_out = dram.tile([NB * P, 512], BF16, tag=f"a2aout{db}",
                                 name=f"a2aout_{rep}")
        self.xT_r = ins["xT"].rearrange("p (bb c u) -> p bb c u",
                                        bb=NB, c=cfg.CC)

        # weights on the gpsimd (SWDGE) queue, emitted at rep start so
        # they dispatch AFTER the previous rep's AllToAll (emitting them
        # earlier would delay the A2A dispatch on this serial queue).
        nc.sync.dma_start(self.mask_sb[:], ins["masks"])
        nc.gpsimd.dma_start(
            self.wqk_sb[:], ins["wqk"].rearrange("p (c m) -> p c m",
                                                 c=cfg.CC))
        nc.gpsimd.dma_start(
            self.wv_sb[:], ins["wv"].rearrange("p (c m) -> p c m",
                                               c=cfg.CC))
        nc.gpsimd.dma_start(self.bqk_sb[:], ins["bqk"])
        nc.gpsimd.dma_start(self.bv_sb[:], ins["bv"])
        nc.gpsimd.dma_start(
            self.wp_sb[:], ins["wp"].rearrange("p (c n) -> p c n",
                                               c=cfg.CC))
        nc.gpsimd.dma_start(self.bp_sb[:], ins["bp"])

        # constants: ones_row is a singleton written once in rep 0; the
        # vsb ones-columns (cols 64:128 of each head, which replicate
        # the softmax denominator into PSUM partitions 64:128 during the
        # AV matmul) are written once per double-buffer slot — v_half
        # only ever writes cols 0:64.
        self.vsb_h = self.vsb.rearrange("p k (h w) -> p k h w", w=VW)
        if rep == 0:
            state["ones_row"] = persist.tile([1, P], BF16, tag="ones_row",
                                             name="ones_row0")
            nc.vector.memset(state["ones_row"][:], 1.0)
            state["ident"] = persist.tile([P, P], BF16, tag="ident",
                                          name="ident0")
            nc.scalar.dma_start(state["ident"][:], ins["ident"])
        self.ones_row = state["ones_row"]
        self.ident = state["ident"]
        if rep < 2:
            nc.vector.memset(self.vsb_h[:, :, :, 64:VW], 1.0)

    def xt_load(self, bb):
        xt = self.pools["s1"].tile([P, self.cfg.CC, 512], BF16, tag="xt",
                                   bufs=3, name=f"xt{bb}_{self.rep}")
        self.nc.sync.dma_start(xt[:], self.xT_r[:, bb, :, :])
        return xt

    def qk_quarter(self, bb, m, xt, quarter, st):
        nc = self.nc
        CC = self.cfg.CC
        if quarter == 0:
            st["acc"] = self.pools["ps_acc"].tile(
                [P, 512], F32, tag="acc", name=f"qkacc{bb}_{m}_{self.rep}")
        acc = st["acc"]
        for cc in range(quarter * 2, quarter * 2 + 2):
            nc.tensor.matmul(
                acc[:],
                self.wqk_sb[:, cc, m * P:(m + 1) * P],
                xt[:, cc, :],
                start=(cc == 0),
                stop=(cc == CC - 1),
            )
        if quarter == 3:
            nc.vector.tensor_scalar_add(
                self.qkT[:, m, bb * 512:(bb + 1) * 512], acc[:],
                self.bqk_sb[:, m:m + 1],
            )

    def v_big(self, bb, xt, half, st):
        # v^T [128 v-dims, 512 t] dim-major: 8 matmuls per block instead
        # of 32 token-major ones; transposed below on the PE.
        nc = self.nc
        if half == 0:
            st["vT"] = self.pools["ps_acc"].tile(
                [P, 512], F32, tag="acc", name=f"vT{bb}_{self.rep}")
        for cc in range(half * 4, half * 4 + 4):
            nc.tensor.matmul(
                st["vT"][:],
                self.wv_sb[:, cc, :],
                xt[:, cc, :],
                start=(cc == 0),
                stop=(cc == self.cfg.CC - 1),
            )
        if half == 1:
            vt_sb = self.pools["s2"].tile([P, 512], BF16, tag="vts",
                                          bufs=2, name=f"vts{bb}_{self.rep}")
            nc.scalar.activation(vt_sb[:], st["vT"][:], COPY)
            st["vt_sb"] = vt_sb

    def v_tr(self, bb, ts, st):
        nc = self.nc
        kt_g = bb * 4 + ts
        if ts == 0:
            st["vtr"] = self.pools["ps_acc"].tile(
                [P, 512], BF16, tag="acc", name=f"vtr{bb}_{self.rep}")
        col = slice(ts * P, (ts + 1) * P)
        nc.tensor.transpose(st["vtr"][:, col], st["vt_sb"][:, col],
                            self.ident[:])
        nc.vector.tensor_copy(
            self.vsb_h[:, kt_g, :, 0:64],
            st["vtr"][:, col].rearrange("p (h d) -> p h d", d=D),
        )

    def stage1_fillers(self, bb):
        xt = self.xt_load(bb)
        fs = []
        for m in range(QKCH):
            st = {}
            for q in range(4):
                fs.append(
                    lambda m=m, q=q, st=st: self.qk_quarter(bb, m, xt, q, st))
        vst = {}
        fs.append(lambda st=vst: self.v_big(bb, xt, 0, st))
        fs.append(lambda st=vst: self.v_big(bb, xt, 1, st))
        for ts in range(4):
            fs.append(lambda ts=ts, st=vst: self.v_tr(bb, ts, st))
        return fs


def emit(tc, outs, ins, cfg):
    from contextlib import ExitStack
    with ExitStack() as _stk:
        pools = dict(
            persist=_stk.enter_context(tc.tile_pool(name="persist", bufs=1)),
            s1=_stk.enter_context(tc.tile_pool(name="s1", bufs=2)),
            s2=_stk.enter_context(tc.tile_pool(name="s2", bufs=4)),
            s3=_stk.enter_context(tc.tile_pool(name="s3", bufs=2)),
            dram=_stk.enter_context(
                tc.tile_pool(name="dram", bufs=1, space="DRAM")),
            ps_acc=_stk.enter_context(tc.tile_pool(
                name="ps_acc", bufs=3, space="PSUM")),
            ps_s=_stk.enter_context(tc.tile_pool(
                name="ps_s", bufs=3, space="PSUM")),
            ps_y=_stk.enter_context(tc.tile_pool(
                name="ps_y", bufs=2, space="PSUM")),
        )
        carry = []
        state = {}
        for rep in range(cfg.repeat):
            carry = _emit_once(tc, outs["out"], ins, cfg, rep, pools, carry,
                               state)
        for f in carry:
            f()


def _emit_once(tc, out, ins, cfg, rep, pools, carry, state):
    nc = tc.nc
    CC = cfg.CC
    ctx = _RepCtx(tc, ins, cfg, rep, pools, state)
    for f in ctx.stage1_fillers(0):
        f()

    s2 = pools["s2"]
    s3 = pools["s3"]
    ps_acc = pools["ps_acc"]
    ps_s = pools["ps_s"]
    ps_y = pools["ps_y"]
    qkT, vsb, mask_sb = ctx.qkT, ctx.vsb, ctx.mask_sb
    ones_row = ctx.ones_row

    # ---- proj chunks (for THIS rep; dispensed during the NEXT rep) ----
    a2a_out_r = ctx.a2a_out.rearrange("(c p) t -> p c t", p=P)

    def proj_load(st):
        # full y^T [1024, 512] for my block: 1KB runs per (p, chunk)
        ag = s3.tile([P, CC, 512], BF16, tag="ag")
        nc.gpsimd.dma_start(ag[:], a2a_out_r[:])
        st["ag"] = ag

    def proj_half(tsub, hh, half, st, pst):
        col = slice(tsub * P, (tsub + 1) * P)
        if half == 0:
            pst["op"] = ps_acc.tile([P, 512], F32, tag="acc",
                                    name=f"pacc{tsub}_{hh}_{rep}")
        op = pst["op"]
        for cc in range(half * 4, half * 4 + 4):
            nc.tensor.matmul(
                op[:], st["ag"][:, cc, col],
                ctx.wp_sb[:, cc, hh * 512:(hh + 1) * 512],
                start=(cc == 0), stop=False,
            )
        if half == 1:
            nc.tensor.matmul(
                op[:], ones_row[:1, :],
                ctx.bp_sb[:1, hh * 512:(hh + 1) * 512],
                start=False, stop=True,
            )
            o_sb = s3.tile([P, 512], F32, tag="osb")
            nc.scalar.activation(o_sb[:], op[:], COPY)
            nc.sync.dma_start(
                out[tsub * P:(tsub + 1) * P, hh * 512:(hh + 1) * 512],
                o_sb[:])

    def proj_fillers():
        st = {}
        fs = [lambda st=st: proj_load(st)]
        for tsub in range(4):
            for hh in range(2):
                pst = {}
                fs.append(lambda tsub=tsub, hh=hh, pst=pst, st=st:
                          proj_half(tsub, hh, 0, st, pst))
                fs.append(lambda tsub=tsub, hh=hh, pst=pst, st=st:
                          proj_half(tsub, hh, 1, st, pst))
        return fs

    # ---- attention with filler dispensing ----
    def attention(bb, early, late, pre):
        # early: fillers safe to run from the start (stage1 of bb+1)
        # late: fillers needing the previous rep's AllToAll (proj)
        # pre: deferred work (previous block's normalize) that MUST be
        #      emitted before this block's first AV (ps_y WAR).
        b, qb = divmod(bb, 4)
        nkt = 4 * qb + 4
        kt_order = list(range(4 * qb, nkt)) + list(range(0, 4 * qb))
        fillers = list(early)
        late = list(late)
        ci = 0  # chain iterations done

        def dispense(n):
            for _ in range(n):
                if fillers:
                    fillers.pop(0)()

        hs = (0, 1)
        pbs = [slice(h * 64, h * 64 + 64) for h in hs]
        ys = [ps_y.tile([P, 512], F32, tag="y",
                        name=f"y{bb}_{h}_{rep}") for h in hs]
        for ki, kt in enumerate(kt_order):
            if ki == 0:
                fillers.extend(late)
                late = []
            j = kt - 4 * qb
            lo = 128 * j if j > 0 else 0
            e = s2.tile([P, 2, 512], BF16, tag="e", bufs=6,
                        name=f"e{bb}_{kt}_{rep}")
            ss = []
            for i in range(2):
                s = ps_s.tile([P, 512], F32, tag="s",
                              name=f"s{bb}_{kt}_{i}_{rep}")
                ss.append(s)
                nc.tensor.matmul(
                    s[:, lo:],
                    qkT[pbs[i], 1,
                        b * cfg.T + kt * P:b * cfg.T + (kt + 1) * P],
                    qkT[pbs[i], 0, bb * 512 + lo:(bb + 1) * 512],
                    start=True, stop=True,
                )
                # per-head exp: half the chain latency of a wide one
                nc.scalar.activation(
                    e[:, i, lo:], s[:, lo:], EXP, scale=0.125)
                if j >= 0:
                    # only the diagonal 128-wide k-tile needs masking;
                    # all farther query columns are fully valid.
                    nc.vector.tensor_mul(
                        e[:, i, lo:lo + P], e[:, i, lo:lo + P],
                        mask_sb[:, j, lo:lo + P])
            if ki == 0:
                for f in pre:
                    f()
                pre = []
            ci += 1
            # dispense fillers between QK and AV: the filler matmuls
            # hide the exp->mask latency on in-order PE.
            rem_work = len(fillers) + len(late)
            rem_iter = nkt - ci
            if rem_iter > 0 and rem_work > 0:
                per = (rem_work + rem_iter - 1) // rem_iter
                dispense(min(per, 2))
            for i in range(2):
                nc.tensor.matmul(
                    ys[i][:, lo:],
                    vsb[:, b * cfg.KT + kt, hs[i] * VW:(hs[i] + 1) * VW],
                    e[:, i, lo:],
                    start=(ki == 0), stop=(ki == nkt - 1),
                )
        for f in pre:
            f()
        fillers.extend(late)
        dispense(len(fillers))
        return [lambda i=i: _normalize(bb, hs[i], ys[i]) for i in (0, 1)]

    def _normalize(bb, h, y):
        # the AV ones-columns replicated the denominator into PSUM
        # partitions 64:128; reciprocal straight into partitions 0:64.
        rec = s2.tile([64, 512], F32R, tag="rec", bufs=3,
                      name=f"rec{bb}_{h}_{rep}")
        with nc.allow_low_precision(
            reason="reciprocal of softmax denominators; ~1e-6"
            " relative is plenty"
        ):
            nc.vector.reciprocal(rec[:], y[64:128, :])
        ynf = s2.tile([64, 512], F32, tag="ynf", bufs=3,
                      name=f"ynf{bb}_{h}_{rep}")
        nc.vector.tensor_mul(ynf[:], y[0:64, :], rec[:])
        yn = s2.tile([64, 512], BF16, tag="yn", bufs=3,
                     name=f"yn{bb}_{h}_{rep}")
        # V bias folded here: y/denom + bv (per-partition scalar)
        nc.vector.tensor_scalar_add(yn[:], ynf[:], ctx.bv_sb[:, h:h + 1])
        nc.sync.dma_start(
            ctx.a2a_in[bb * P + h * D:bb * P + (h + 1) * D, :], yn[:])

    def all_to_all():
        if cfg.fake_collective:
            nc.gpsimd.dma_start(ctx.a2a_out[:], ctx.a2a_in[:])
            return
        nc.gpsimd.collective_compute(
            "AllToAll", mybir.AluOpType.bypass,
            replica_groups=cfg.replica_groups,
            ins=[ctx.a2a_in[:].opt()],
            outs=[ctx.a2a_out[:].opt()],
        )

    # ---- fused pipeline ----
    pre = []
    for bb in range(NB):
        early = ctx.stage1_fillers(bb + 1) if bb + 1 < NB else []
        late = []
        if bb == PROJ_BLOCK - 1 and carry:
            early.append(carry.pop(0))  # the a2a_out load leads by a block
        if bb == PROJ_BLOCK:
            late = list(carry)
            carry = []
        pre = attention(bb, early, late, pre)
    # block 7's normalize must precede the A2A emission
    for f in pre:
        f()
    # leftover carry (repeat=1 case): flush before the collective
    for f in carry:
        f()
    all_to_all()
    # defer this rep's proj (A2A-dependent) into the next rep
    return proj_fillers()


SHAPES = {
    "xT": ((P, NB * (C // P) * 512), BF16),
    "wqk": ((P, (C // P) * QKCH * P), BF16),
    "wv": ((P, (C // P) * HL * D), BF16),
    "bqk": ((P, QKCH), F32),
    "bv": ((D, HL), F32),
    "wp": ((P, (C // P) * C), BF16),
    "bp": ((1, C), BF16),
    "masks": ((P, 4, 512), BF16),
    "ident": ((P, P), BF16),
}


def build(cfg, num_devices=None):
    nc = bacc.Bacc("TRN2", target_bir_lowering=False, debug=False,
                   num_devices=num_devices or cfg.n_cores)
    ins = {}
    for name, (shape, dt) in SHAPES.items():
        ins[name] = nc.dram_tensor(
            name, list(shape), dt, kind="ExternalInput").ap()
    outs = {"out": nc.dram_tensor(
        "out", [512, C], F32, kind="ExternalOutput").ap()}
    with tile.TileContext(nc) as tc:
        emit(tc, outs, ins, cfg)
    nc.compile()
    return nc


def make_core_inputs(x_full, c_attn_w, c_attn_b, c_proj_w, c_proj_b, cfg,
                     core):
    import ml_dtypes
    bf = ml_dtypes.bfloat16
    T = cfg.T
    hs = slice(core * HL * D, (core + 1) * HL * D)
    wq = c_attn_w[:, 0 * C:1 * C][:, hs]
    wk = c_attn_w[:, 1 * C:2 * C][:, hs]
    wv = c_attn_w[:, 2 * C:3 * C][:, hs]
    bq = c_attn_b[0 * C:1 * C][hs]
    bk = c_attn_b[1 * C:2 * C][hs]
    bv = c_attn_b[2 * C:3 * C][hs]

    pp = np.arange(P)[:, None, None]
    jj = np.arange(4)[None, :, None]
    qq = np.arange(512)[None, None, :]
    masks = (qq >= pp + 128 * jj)

    def chunkp(w):
        # [C, m] -> [P, CC*m]: row p holds chunk-major contiguous runs,
        # so every SBUF load is one long run per partition.
        m = w.shape[1]
        return np.ascontiguousarray(
            w.reshape(C // P, P, m).transpose(1, 0, 2).reshape(P, -1)
            .astype(bf))

    # x^T for BOTH batches, block-major (bb = b*TB + tb):
    # [P, bb, cc, 512]
    xbs = []
    for b in range(B):
        xt = x_full[b, :T].T  # [C, T]
        xbs.append(xt.reshape(C // P, P, T // 512, 512)
                   .transpose(1, 2, 0, 3))  # [P, tb, cc, 512]
    xT2 = np.concatenate(xbs, axis=1).reshape(P, -1)

    return {
        "xT": np.ascontiguousarray(xT2.astype(bf)),
        "wqk": chunkp(np.concatenate([wq, wk], axis=1)),
        "wv": chunkp(wv),
        "bqk": np.ascontiguousarray(
            np.concatenate([bq, bk]).reshape(QKCH, P).T, np.float32),
        "bv": np.ascontiguousarray(
            bv.reshape(HL, D).T, np.float32),
        "wp": chunkp(c_proj_w),
        "bp": np.ascontiguousarray(c_proj_b[None, :].astype(bf)),
        "masks": masks.astype(bf),
        "ident": np.eye(P, dtype=bf),
    }


_CACHE = {}


def kernel(**inputs):
    from concourse.bass_utils import run_bass_kernel_spmd

    cfg = CFG_FULL
    x = np.asarray(inputs["x"], np.float32)
    c_attn_w = np.asarray(inputs["c_attn_w"], np.float32)
    c_attn_b = np.asarray(inputs["c_attn_b"], np.float32)
    c_proj_w = np.asarray(inputs["c_proj_w"], np.float32)
    c_proj_b = np.asarray(inputs["c_proj_b"], np.float32)

    if "nc" not in _CACHE:
        _CACHE["nc"] = build(cfg)
    nc = _CACHE["nc"]
    in_maps = [
        make_core_inputs(x, c_attn_w, c_attn_b, c_proj_w, c_proj_b, cfg,
                         core)
        for core in range(cfg.n_cores)
    ]
    res = run_bass_kernel_spmd(nc, in_maps, core_ids=list(range(cfg.n_cores)))
    out = np.empty((B, T_FULL, C), np.float32)
    for core in range(cfg.n_cores):
        b, tb = divmod(core, cfg.TB)
        out[b, tb * 512:(tb + 1) * 512, :] = res.results[core]["out"]
    return out


# revision 12
# speedup vs baseline: 1.1016x; 1.0407x over previous
"""GPT-2 style multi-head attention on 8 Trainium2 cores (Bass/Tile), v3.

Problem: B=2, T=2048, C=1024, H=16 heads, D=64, fp32 in/out.

Sharding (hardcoded): 8-way head-parallel. Core c computes heads
[2c, 2c+2) for BOTH batches (8 blocks of 512 tokens each, bb = b*4+tb).
After attention, ONE AllToAll per rep reshards y^T from head-sharded to
t-block-sharded: rank d ends up with the full [1024, 512] y^T for block
d and computes the full-width output projection locally (full c_proj_w,
full bias). Core d returns out rows [512*tb_d, 512*tb_d+512) of batch
b_d.

v3 vs v2 (which AllGathered y per q-block, 8 collectives/rep): the
collective queue was the bottleneck — each collective costs ~15us launch
overhead + transfer, serialized. One 1MB AllToAll (~40us) hides under
~125us of PE work.

Pipeline structure:
  - host ships x^T (c-major) in bf16: no PE transposes.
  - all weights/activations bf16 (PSUM accum fp32).
  - wide exp: one ACT instruction covers both heads' scores; the causal
    mask multiply only touches the 128-wide diagonal k-tile.
  - V bias folded into the normalize (yn = y*rec + bv as a per-partition
    scalar add) instead of a PE bias matmul.
  - software pipelining: stage1(bb+1) and proj(rep-1) matmul chunks are
    dispensed as fillers between attention(bb) chain iterations; the
    NEXT rep's tiles + weight loads + stage1(block 0) are created at
    block 7 so the in-order PE never drains at the rep boundary.
  - DMA queues by dependency class: SP = input streams + compute-near
    writes; gpsimd(SWDGE) = weights, collective, and A2A-dependent
    proj loads.
  - persistent tensors double-buffered across timing reps (tag rep%2).
"""

import numpy as np

import concourse.bass as bass
import concourse.mybir as mybir
import concourse.tile as tile
from concourse import bacc

P = 128
B, T_FULL, C, H, D = 2, 2048, 1024, 16, 64
F32 = mybir.dt.float32
F32R = mybir.dt.float32r
BF16 = mybir.dt.bfloat16
EXP = mybir.ActivationFunctionType.Exp
COPY = mybir.ActivationFunctionType.Copy
VW = 128  # per-head V stride: 64 V cols + 64 ones cols
HL = 2   # heads per core
NB = 8   # (batch, t-block) blocks
QKCH = 2  # qkT m-chunks: 0=Q, 1=K


class Cfg:
    def __init__(self, n_cores, group_size, T, fake_collective=False,
                 repeat=1):
        self.fake_collective = fake_collective
        self.repeat = repeat
        self.n_cores = n_cores
        self.T = T
        self.HL = HL
        self.CC = C // P                   # contraction chunks (8)
        self.TB = T // 512                 # t-blocks per batch (4)
        self.KT = T // P                   # k tiles per batch (16)
        if n_cores == 8:
            self.replica_groups = [[0, 1, 2, 3, 4, 5, 6, 7]]
        elif n_cores == 1:
            self.replica_groups = [[0]]
        else:
            raise ValueError(n_cores)


CFG_FULL = Cfg(8, 4, T_FULL)

# block at which the previous rep's proj fillers (dependent on the
# previous rep's AllToAll) are dispensed. Block 7 has no stage1 fillers
# of its own (there is no block 8), and the A2A is long done by then.
PROJ_BLOCK = 7


class _RepCtx:
    """Per-rep tiles + stage1 closures. Created one rep AHEAD (at block
    7 of the previous rep) so the next rep's weight DMAs are queued
    before the previous rep's AllToAll and its stage1(0) can fill the
    previous rep's last attention block."""

    def __init__(self, tc, ins, cfg, rep, pools, state):
        nc = tc.nc
        self.nc = nc
        self.cfg = cfg
        self.rep = rep
        self.pools = pools
        db = rep % 2
        persist = pools["persist"]
        dram = pools["dram"]
        s1 = pools["s1"]

        self.qkT = persist.tile([P, QKCH, NB * 512], BF16, tag=f"qkT{db}",
                                name=f"qkT_{rep}")
        self.vsb = persist.tile([P, 2 * cfg.KT, HL * VW], BF16,
                                tag=f"vsb{db}", name=f"vsb_{rep}")
        self.mask_sb = persist.tile([P, 4, 512], BF16, tag=f"mask{db}",
                                    name=f"mask_{rep}")
        self.wqk_sb = persist.tile([P, cfg.CC, QKCH * P], BF16,
                                   tag=f"wqk{db}", name=f"wqk_{rep}")
        self.wv_sb = persist.tile([P, cfg.CC, HL * D], BF16,
                                  tag=f"wv{db}", name=f"wv_{rep}")
        self.wp_sb = persist.tile([P, cfg.CC, C], BF16, tag=f"wp{db}",
                                  name=f"wp_{rep}")
        self.bqk_sb = persist.tile([P, QKCH], F32, tag=f"bqk{db}",
                                   name=f"bqk_{rep}")
        self.bp_sb = persist.tile([1, C], BF16, tag=f"bp{db}",
                                  name=f"bp_{rep}")
        self.a2a_in = dram.tile([NB * P, 512], BF16, tag=f"a2ain{db}",
                                name=f"a2ain_{rep}")
        self.a2a_out = dram.tile([NB * P, 512], BF16, tag=f"a2aout{db}",
                                 name=f"a2aout_{rep}")
        self.xT_r = ins["xT"].rearrange("p (bb c u) -> p bb c u",
                                        bb=NB, c=cfg.CC)

        # weights on the gpsimd (SWDGE) queue, emitted at rep start so
        # they dispatch AFTER the previous rep's AllToAll (emitting them
        # earlier would delay the A2A dispatch on this serial queue).
        nc.sync.dma_start(self.mask_sb[:], ins["masks"])
        nc.gpsimd.dma_start(
            self.wqk_sb[:], ins["wqk"].rearrange("p (c m) -> p c m",
                                                 c=cfg.CC))
        nc.gpsimd.dma_start(
            self.wv_sb[:], ins["wv"].rearrange("p (c m) -> p c m",
                                               c=cfg.CC))
        nc.gpsimd.dma_start(self.bqk_sb[:], ins["bqk"])
        nc.gpsimd.dma_start(
            self.wp_sb[:], ins["wp"].rearrange("p (c n) -> p c n",
                                               c=cfg.CC))
        nc.gpsimd.dma_start(self.bp_sb[:], ins["bp"])

        # constants: ones_row is a singleton written once in rep 0; the
        # vsb ones-columns (cols 64:128 of each head, which replicate
        # the softmax denominator into PSUM partitions 64:128 during the
        # AV matmul) are written once per double-buffer slot — v_half
        # only ever writes cols 0:64.
        self.vsb_h = self.vsb.rearrange("p k (h w) -> p k h w", w=VW)
        if rep == 0:
            state["ones_row"] = persist.tile([1, P], BF16, tag="ones_row",
                                             name="ones_row0")
            nc.vector.memset(state["ones_row"][:], 1.0)
            state["ident"] = persist.tile([P, P], BF16, tag="ident",
                                          name="ident0")
            nc.scalar.dma_start(state["ident"][:], ins["ident"])
        self.ones_row = state["ones_row"]
        self.ident = state["ident"]
        if rep < 2:
            nc.vector.memset(self.vsb_h[:, :, :, 64:VW], 1.0)

    def xt_load(self, bb):
        xt = self.pools["s1"].tile([P, self.cfg.CC, 512], BF16, tag="xt",
                                   bufs=3, name=f"xt{bb}_{self.rep}")
        self.nc.sync.dma_start(xt[:], self.xT_r[:, bb, :, :])
        return xt

    def qk_quarter(self, bb, m, xt, quarter, st):
        nc = self.nc
        CC = self.cfg.CC
        if quarter == 0:
            st["acc"] = self.pools["ps_acc"].tile(
                [P, 512], F32, tag="acc", name=f"qkacc{bb}_{m}_{self.rep}")
        acc = st["acc"]
        for cc in range(quarter * 2, quarter * 2 + 2):
            nc.tensor.matmul(
                acc[:],
                self.wqk_sb[:, cc, m * P:(m + 1) * P],
                xt[:, cc, :],
                start=(cc == 0),
                stop=(cc == CC - 1),
            )
        if quarter == 3:
            nc.vector.tensor_scalar_add(
                self.qkT[:, m, bb * 512:(bb + 1) * 512], acc[:],
                self.bqk_sb[:, m:m + 1],
            )

    def v_big(self, bb, xt, half, st):
        # v^T [128 v-dims, 512 t] dim-major: 8 matmuls per block instead
        # of 32 token-major ones; transposed below on the PE.
        nc = self.nc
        if half == 0:
            st["vT"] = self.pools["ps_acc"].tile(
                [P, 512], F32, tag="acc", name=f"vT{bb}_{self.rep}")
        for cc in range(half * 4, half * 4 + 4):
            nc.tensor.matmul(
                st["vT"][:],
                self.wv_sb[:, cc, :],
                xt[:, cc, :],
                start=(cc == 0),
                stop=(cc == self.cfg.CC - 1),
            )
        if half == 1:
            vt_sb = self.pools["s2"].tile([P, 512], BF16, tag="vts",
                                          bufs=2, name=f"vts{bb}_{self.rep}")
            nc.scalar.activation(vt_sb[:], st["vT"][:], COPY)
            st["vt_sb"] = vt_sb

    def v_tr(self, bb, ts, st):
        nc = self.nc
        kt_g = bb * 4 + ts
        if ts == 0:
            st["vtr"] = self.pools["ps_acc"].tile(
                [P, 512], BF16, tag="acc", name=f"vtr{bb}_{self.rep}")
        col = slice(ts * P, (ts + 1) * P)
        nc.tensor.transpose(st["vtr"][:, col], st["vt_sb"][:, col],
                            self.ident[:])
        nc.vector.tensor_copy(
            self.vsb_h[:, kt_g, :, 0:64],
            st["vtr"][:, col].rearrange("p (h d) -> p h d", d=D),
        )

    def stage1_fillers(self, bb):
        xt = self.xt_load(bb)
        fs = []
        for m in range(QKCH):
            st = {}
            for q in range(4):
                fs.append(
                    lambda m=m, q=q, st=st: self.qk_quarter(bb, m, xt, q, st))
        vst = {}
        fs.append(lambda st=vst: self.v_big(bb, xt, 0, st))
        fs.append(lambda st=vst: self.v_big(bb, xt, 1, st))
        for ts in range(4):
            fs.append(lambda ts=ts, st=vst: self.v_tr(bb, ts, st))
        return fs


def emit(tc, outs, ins, cfg):
    from contextlib import ExitStack
    with ExitStack() as _stk:
        pools = dict(
            persist=_stk.enter_context(tc.tile_pool(name="persist", bufs=1)),
            s1=_stk.enter_context(tc.tile_pool(name="s1", bufs=2)),
            s2=_stk.enter_context(tc.tile_pool(name="s2", bufs=4)),
            s3=_stk.enter_context(tc.tile_pool(name="s3", bufs=2)),
            dram=_stk.enter_context(
                tc.tile_pool(name="dram", bufs=1, space="DRAM")),
            ps_acc=_stk.enter_context(tc.tile_pool(
                name="ps_acc", bufs=2, space="PSUM")),
            ps_s=_stk.enter_context(tc.tile_pool(
                name="ps_s", bufs=2, space="PSUM")),
            ps_y=_stk.enter_context(tc.tile_pool(
                name="ps_y", bufs=2, space="PSUM")),
        )
        carry = []
        state = {}
        for rep in range(cfg.repeat):
            carry = _emit_once(tc, outs["out"], ins, cfg, rep, pools, carry,
                               state)
        for f in carry:
            f()


def _emit_once(tc, out, ins, cfg, rep, pools, carry, state):
    nc = tc.nc
    CC = cfg.CC
    ctx = _RepCtx(tc, ins, cfg, rep, pools, state)
    for f in ctx.stage1_fillers(0):
        f()

    s2 = pools["s2"]
    s3 = pools["s3"]
    ps_acc = pools["ps_acc"]
    ps_s = pools["ps_s"]
    ps_y = pools["ps_y"]
    qkT, vsb, mask_sb = ctx.qkT, ctx.vsb, ctx.mask_sb
    ones_row = ctx.ones_row

    # ---- proj chunks (for THIS rep; dispensed during the NEXT rep) ----
    a2a_out_r = ctx.a2a_out.rearrange("(c p) t -> p c t", p=P)

    def proj_load(st):
        # full y^T [1024, 512] for my block: 1KB runs per (p, chunk)
        ag = s3.tile([P, CC, 512], BF16, tag="ag")
        nc.gpsimd.dma_start(ag[:], a2a_out_r[:])
        st["ag"] = ag

    def proj_half(tsub, hh, half, st, pst):
        col = slice(tsub * P, (tsub + 1) * P)
        if half == 0:
            pst["op"] = ps_acc.tile([P, 512], F32, tag="acc",
                                    name=f"pacc{tsub}_{hh}_{rep}")
        op = pst["op"]
        for cc in range(half * 4, half * 4 + 4):
            nc.tensor.matmul(
                op[:], st["ag"][:, cc, col],
                ctx.wp_sb[:, cc, hh * 512:(hh + 1) * 512],
                start=(cc == 0), stop=False,
            )
        if half == 1:
            nc.tensor.matmul(
                op[:], ones_row[:1, :],
                ctx.bp_sb[:1, hh * 512:(hh + 1) * 512],
                start=False, stop=True,
            )
            o_sb = s3.tile([P, 512], F32, tag="osb")
            nc.scalar.activation(o_sb[:], op[:], COPY)
            nc.sync.dma_start(
                out[tsub * P:(tsub + 1) * P, hh * 512:(hh + 1) * 512],
                o_sb[:])

    def proj_fillers():
        st = {}
        fs = [lambda st=st: proj_load(st)]
        for tsub in range(4):
            for hh in range(2):
                pst = {}
                fs.append(lambda tsub=tsub, hh=hh, pst=pst, st=st:
                          proj_half(tsub, hh, 0, st, pst))
                fs.append(lambda tsub=tsub, hh=hh, pst=pst, st=st:
                          proj_half(tsub, hh, 1, st, pst))
        return fs

    # ---- attention with filler dispensing ----
    def attention(bb, early, late, pre):
        # early: fillers safe to run from the start (stage1 of bb+1)
        # late: fillers needing the previous rep's AllToAll (proj)
        # pre: deferred work (previous block's normalize) that MUST be
        #      emitted before this block's first AV (ps_y WAR).
        b, qb = divmod(bb, 4)
        nkt = 4 * qb + 4
        kt_order = list(range(4 * qb, nkt)) + list(range(0, 4 * qb))
        fillers = list(early)
        late = list(late)
        ci = 0  # chain iterations done

        def dispense(n):
            for _ in range(n):
                if fillers:
                    fillers.pop(0)()

        hs = (0, 1)
        pbs = [slice(h * 64, h * 64 + 64) for h in hs]
        ys = [ps_y.tile([P, 512], F32, tag="y",
                        name=f"y{bb}_{h}_{rep}") for h in hs]
        # iteration list: 4 diagonal singles, then off-diagonal k-tiles
        # in PAIRS — one exp instruction covers both tiles of a head,
        # cutting ACT instruction count and chain sync hops ~30%.
        items = [("d", kt) for kt in range(4 * qb, nkt)]
        items += [("p", kt) for kt in range(0, 4 * qb, 2)]
        n_it = len(items)
        Kbase = b * cfg.T

        def qk_mm(dst, i, kt, lo):
            nc.tensor.matmul(
                dst,
                qkT[pbs[i], 1, Kbase + kt * P:Kbase + (kt + 1) * P],
                qkT[pbs[i], 0, bb * 512 + lo:(bb + 1) * 512],
                start=True, stop=True,
            )

        def av_mm(i, kt, mov, lo, start, stop):
            nc.tensor.matmul(
                ys[i][:, lo:],
                vsb[:, b * cfg.KT + kt, hs[i] * VW:(hs[i] + 1) * VW],
                mov,
                start=start, stop=stop,
            )

        for ki, (kind, kt) in enumerate(items):
            if ki == 0:
                fillers.extend(late)
                late = []
            first, last = ki == 0, ki == n_it - 1
            e = s2.tile([P, 2, 2, 512], BF16, tag="e", bufs=4,
                        name=f"e{bb}_{kt}_{rep}")
            if kind == "d":
                j = kt - 4 * qb
                lo = 128 * j if j > 0 else 0
                s = ps_s.tile([P, 2, 512], F32, tag="s",
                              name=f"s{bb}_{kt}_{rep}")
                for i in range(2):
                    qk_mm(s[:, i, lo:], i, kt, lo)
                    nc.scalar.activation(
                        e[:, i, 0, lo:], s[:, i, lo:], EXP, scale=0.125)
                    # only the diagonal 128-wide k-tile needs masking
                    nc.vector.tensor_mul(
                        e[:, i, 0, lo:lo + P], e[:, i, 0, lo:lo + P],
                        mask_sb[:, j, lo:lo + P])
            else:
                lo = 0
                for i in range(2):
                    s = ps_s.tile([P, 2, 512], F32, tag="s",
                                  name=f"s{bb}_{kt}_{i}_{rep}")
                    qk_mm(s[:, 0, :], i, kt, 0)
                    qk_mm(s[:, 1, :], i, kt + 1, 0)
                    nc.scalar.activation(
                        e[:, i, :, :], s[:, :, :], EXP, scale=0.125)
            if ki == 0:
                for f in pre:
                    f()
                pre = []
            ci += 1
            # dispense fillers between QK and AV: the filler matmuls
            # hide the exp latency on the in-order PE.
            rem_work = len(fillers) + len(late)
            rem_iter = n_it - ci
            cap = 2 if kind == "d" else 3
            if rem_iter > 0 and rem_work > 0:
                per = (rem_work + rem_iter - 1) // rem_iter
                dispense(min(per, cap))
            for i in range(2):
                av_mm(i, kt, e[:, i, 0, lo:], lo, first, last and kind == "d")
                if kind == "p":
                    av_mm(i, kt + 1, e[:, i, 1, :], 0, False, last)
        for f in pre:
            f()
        fillers.extend(late)
        dispense(len(fillers))
        return [lambda i=i: _normalize(bb, hs[i], ys[i]) for i in (0, 1)]

    def _normalize(bb, h, y):
        # the AV ones-columns replicated the denominator into PSUM
        # partitions 64:128; reciprocal straight into partitions 0:64.
        rec = s2.tile([64, 512], F32R, tag="rec", bufs=3,
                      name=f"rec{bb}_{h}_{rep}")
        with nc.allow_low_precision(
            reason="reciprocal of softmax denominators; ~1e-6"
            " relative is plenty"
        ):
            nc.vector.reciprocal(rec[:], y[64:128, :])
        yn = s2.tile([64, 512], BF16, tag="yn", bufs=3,
                     name=f"yn{bb}_{h}_{rep}")
        nc.vector.tensor_mul(yn[:], y[0:64, :], rec[:])
        # V bias is folded into bp on the host (softmax rows sum to 1,
        # so y_final = yhat + bv and bv@Wp + bp replaces bp).
        nc.sync.dma_start(
            ctx.a2a_in[bb * P + h * D:bb * P + (h + 1) * D, :], yn[:])

    def all_to_all():
        if cfg.fake_collective:
            nc.gpsimd.dma_start(ctx.a2a_out[:], ctx.a2a_in[:])
            return
        nc.gpsimd.collective_compute(
            "AllToAll", mybir.AluOpType.bypass,
            replica_groups=cfg.replica_groups,
            ins=[ctx.a2a_in[:].opt()],
            outs=[ctx.a2a_out[:].opt()],
        )

    # ---- fused pipeline ----
    pre = []
    for bb in range(NB):
        early = ctx.stage1_fillers(bb + 1) if bb + 1 < NB else []
        late = []
        if bb == PROJ_BLOCK - 1 and carry:
            early.append(carry.pop(0))  # the a2a_out load leads by a block
        if bb == PROJ_BLOCK:
            late = list(carry)
            carry = []
        pre = attention(bb, early, late, pre)
    # block 7's normalize must precede the A2A emission
    for f in pre:
        f()
    # leftover carry (repeat=1 case): flush before the collective
    for f in carry:
        f()
    all_to_all()
    # defer this rep's proj (A2A-dependent) into the next rep
    return proj_fillers()


SHAPES = {
    "xT": ((P, NB * (C // P) * 512), BF16),
    "wqk": ((P, (C // P) * QKCH * P), BF16),
    "wv": ((P, (C // P) * HL * D), BF16),
    "bqk": ((P, QKCH), F32),
    "wp": ((P, (C // P) * C), BF16),
    "bp": ((1, C), BF16),
    "masks": ((P, 4, 512), BF16),
    "ident": ((P, P), BF16),
}


def build(cfg, num_devices=None):
    nc = bacc.Bacc("TRN2", target_bir_lowering=False, debug=False,
                   num_devices=num_devices or cfg.n_cores)
    ins = {}
    for name, (shape, dt) in SHAPES.items():
        ins[name] = nc.dram_tensor(
            name, list(shape), dt, kind="ExternalInput").ap()
    outs = {"out": nc.dram_tensor(
        "out", [512, C], F32, kind="ExternalOutput").ap()}
    with tile.TileContext(nc) as tc:
        emit(tc, outs, ins, cfg)
    nc.compile()
    return nc


def make_core_inputs(x_full, c_attn_w, c_attn_b, c_proj_w, c_proj_b, cfg,
                     core):
    import ml_dtypes
    bf = ml_dtypes.bfloat16
    T = cfg.T
    hs = slice(core * HL * D, (core + 1) * HL * D)
    wq = c_attn_w[:, 0 * C:1 * C][:, hs]
    wk = c_attn_w[:, 1 * C:2 * C][:, hs]
    wv = c_attn_w[:, 2 * C:3 * C][:, hs]
    bq = c_attn_b[0 * C:1 * C][hs]
    bk = c_attn_b[1 * C:2 * C][hs]
    bv_full = c_attn_b[2 * C:3 * C]
    bp_eff = c_proj_b + bv_full @ c_proj_w

    pp = np.arange(P)[:, None, None]
    jj = np.arange(4)[None, :, None]
    qq = np.arange(512)[None, None, :]
    masks = (qq >= pp + 128 * jj)

    def chunkp(w):
        # [C, m] -> [P, CC*m]: row p holds chunk-major contiguous runs,
        # so every SBUF load is one long run per partition.
        m = w.shape[1]
        return np.ascontiguousarray(
            w.reshape(C // P, P, m).transpose(1, 0, 2).reshape(P, -1)
            .astype(bf))

    # x^T for BOTH batches, block-major (bb = b*TB + tb):
    # [P, bb, cc, 512]
    xbs = []
    for b in range(B):
        xt = x_full[b, :T].T  # [C, T]
        xbs.append(xt.reshape(C // P, P, T // 512, 512)
                   .transpose(1, 2, 0, 3))  # [P, tb, cc, 512]
    xT2 = np.concatenate(xbs, axis=1).reshape(P, -1)

    return {
        "xT": np.ascontiguousarray(xT2.astype(bf)),
        "wqk": chunkp(np.concatenate([wq, wk], axis=1)),
        "wv": chunkp(wv),
        "bqk": np.ascontiguousarray(
            np.concatenate([bq, bk]).reshape(QKCH, P).T, np.float32),
        "wp": chunkp(c_proj_w),
        "bp": np.ascontiguousarray(bp_eff[None, :].astype(bf)),
        "masks": masks.astype(bf),
        "ident": np.eye(P, dtype=bf),
    }


_CACHE = {}


def kernel(**inputs):
    from concourse.bass_utils import run_bass_kernel_spmd

    cfg = CFG_FULL
    x = np.asarray(inputs["x"], np.float32)
    c_attn_w = np.asarray(inputs["c_attn_w"], np.float32)
    c_attn_b = np.asarray(inputs["c_attn_b"], np.float32)
    c_proj_w = np.asarray(inputs["c_proj_w"], np.float32)
    c_proj_b = np.asarray(inputs["c_proj_b"], np.float32)

    if "nc" not in _CACHE:
        _CACHE["nc"] = build(cfg)
    nc = _CACHE["nc"]
    in_maps = [
        make_core_inputs(x, c_attn_w, c_attn_b, c_proj_w, c_proj_b, cfg,
                         core)
        for core in range(cfg.n_cores)
    ]
    res = run_bass_kernel_spmd(nc, in_maps, core_ids=list(range(cfg.n_cores)))
    out = np.empty((B, T_FULL, C), np.float32)
    for core in range(cfg.n_cores):
        b, tb = divmod(core, cfg.TB)
        out[b, tb * 512:(tb + 1) * 512, :] = res.results[core]["out"]
    return out


# revision 14
# speedup vs baseline: 1.1172x; 1.0141x over previous
"""GPT-2 style multi-head attention on 8 Trainium2 cores (Bass/Tile), v3.

Problem: B=2, T=2048, C=1024, H=16 heads, D=64, fp32 in/out.

Sharding (hardcoded): 8-way head-parallel. Core c computes heads
[2c, 2c+2) for BOTH batches (8 blocks of 512 tokens each, bb = b*4+tb).
After attention, ONE AllToAll per rep reshards y^T from head-sharded to
t-block-sharded: rank d ends up with the full [1024, 512] y^T for block
d and computes the full-width output projection locally (full c_proj_w,
full bias). Core d returns out rows [512*tb_d, 512*tb_d+512) of batch
b_d.

v3 vs v2 (which AllGathered y per q-block, 8 collectives/rep): the
collective queue was the bottleneck — each collective costs ~15us launch
overhead + transfer, serialized. One 1MB AllToAll (~40us) hides under
~125us of PE work.

Pipeline structure:
  - host ships x^T (c-major) in bf16: no PE transposes.
  - all weights/activations bf16 (PSUM accum fp32).
  - off-diagonal k-tiles processed in PAIRS: one exp instruction per
    head covers two tiles; the causal mask multiply only touches the
    128-wide diagonal k-tile. The softmax denominator is replicated
    into PSUM partitions 64:128 by ones-columns in the AV stationary,
    so normalize is reciprocal+mul only (no broadcast matmul).
  - V bias folded into the projection bias on the host (softmax rows
    sum to 1, so bp_eff = bp + bv @ c_proj_w).
  - software pipelining: stage1(bb+1) and proj(rep-1) matmul chunks are
    dispensed as fillers between attention(bb) chain iterations; the
    NEXT rep's tiles + weight loads + stage1(block 0) are created at
    block 7 so the in-order PE never drains at the rep boundary.
  - DMA queues by dependency class: SP = input streams + compute-near
    writes; gpsimd(SWDGE) = weights, collective, and A2A-dependent
    proj loads.
  - persistent tensors double-buffered across timing reps (tag rep%2).
"""

import numpy as np

import concourse.bass as bass
import concourse.mybir as mybir
import concourse.tile as tile
from concourse import bacc

P = 128
B, T_FULL, C, H, D = 2, 2048, 1024, 16, 64
F32 = mybir.dt.float32
F32R = mybir.dt.float32r
BF16 = mybir.dt.bfloat16
EXP = mybir.ActivationFunctionType.Exp
COPY = mybir.ActivationFunctionType.Copy
VW = 128  # per-head V stride: 64 V cols + 64 ones cols
HL = 2   # heads per core
NB = 8   # (batch, t-block) blocks
QKCH = 2  # qkT m-chunks: 0=Q, 1=K


class Cfg:
    def __init__(self, n_cores, group_size, T, fake_collective=False,
                 repeat=1):
        self.fake_collective = fake_collective
        self.repeat = repeat
        self.n_cores = n_cores
        self.T = T
        self.HL = HL
        self.CC = C // P                   # contraction chunks (8)
        self.TB = T // 512                 # t-blocks per batch (4)
        self.KT = T // P                   # k tiles per batch (16)
        if n_cores == 8:
            self.replica_groups = [[0, 1, 2, 3, 4, 5, 6, 7]]
        elif n_cores == 1:
            self.replica_groups = [[0]]
        else:
            raise ValueError(n_cores)


CFG_FULL = Cfg(8, 4, T_FULL)

# block at which the previous rep's proj fillers (dependent on the
# previous rep's AllToAll) are dispensed. Block 7 has no stage1 fillers
# of its own (there is no block 8), and the A2A is long done by then.
PROJ_BLOCK = 7


class _RepCtx:
    """Per-rep tiles + stage1 closures. Created one rep AHEAD (at block
    7 of the previous rep) so the next rep's weight DMAs are queued
    before the previous rep's AllToAll and its stage1(0) can fill the
    previous rep's last attention block."""

    def __init__(self, tc, ins, cfg, rep, pools, state):
        nc = tc.nc
        self.nc = nc
        self.cfg = cfg
        self.rep = rep
        self.pools = pools
        db = rep % 2
        persist = pools["persist"]
        dram = pools["dram"]
        s1 = pools["s1"]

        self.qkT = persist.tile([P, QKCH, NB * 512], BF16, tag=f"qkT{db}",
                                name=f"qkT_{rep}")
        self.vsb = persist.tile([P, 2 * cfg.KT, HL * VW], BF16,
                                tag=f"vsb{db}", name=f"vsb_{rep}")
        self.mask_sb = persist.tile([P, 4, 512], BF16, tag=f"mask{db}",
                                    name=f"mask_{rep}")
        self.wqk_sb = persist.tile([P, cfg.CC, QKCH * P], BF16,
                                   tag=f"wqk{db}", name=f"wqk_{rep}")
        self.wv_sb = persist.tile([P, cfg.CC, HL * D], BF16,
                                  tag=f"wv{db}", name=f"wv_{rep}")
        self.wp_sb = persist.tile([P, cfg.CC, C], BF16, tag=f"wp{db}",
                                  name=f"wp_{rep}")
        self.bqk_sb = persist.tile([P, QKCH], F32, tag=f"bqk{db}",
                                   name=f"bqk_{rep}")
        self.bp_sb = persist.tile([1, C], BF16, tag=f"bp{db}",
                                  name=f"bp_{rep}")
        self.a2a_in = dram.tile([NB * P, 512], BF16, tag=f"a2ain{db}",
                                name=f"a2ain_{rep}")
        self.a2a_out = dram.tile([NB * P, 512], BF16, tag=f"a2aout{db}",
                                 name=f"a2aout_{rep}")
        self.xT_r = ins["xT"].rearrange("p (bb c u) -> p bb c u",
                                        bb=NB, c=cfg.CC)

        # weights on the gpsimd (SWDGE) queue, emitted at rep start so
        # they dispatch AFTER the previous rep's AllToAll (emitting them
        # earlier would delay the A2A dispatch on this serial queue).
        nc.sync.dma_start(self.mask_sb[:], ins["masks"])
        nc.gpsimd.dma_start(
            self.wqk_sb[:], ins["wqk"].rearrange("p (c m) -> p c m",
                                                 c=cfg.CC))
        nc.gpsimd.dma_start(
            self.wv_sb[:], ins["wv"].rearrange("p (c m) -> p c m",
                                               c=cfg.CC))
        nc.gpsimd.dma_start(self.bqk_sb[:], ins["bqk"])
        nc.gpsimd.dma_start(
            self.wp_sb[:], ins["wp"].rearrange("p (c n) -> p c n",
                                               c=cfg.CC))
        nc.gpsimd.dma_start(self.bp_sb[:], ins["bp"])

        # constants: ones_row is a singleton written once in rep 0; the
        # vsb ones-columns (cols 64:128 of each head, which replicate
        # the softmax denominator into PSUM partitions 64:128 during the
        # AV matmul) are written once per double-buffer slot — v_half
        # only ever writes cols 0:64.
        self.vsb_h = self.vsb.rearrange("p k (h w) -> p k h w", w=VW)
        if rep == 0:
            state["ones_row"] = persist.tile([1, P], BF16, tag="ones_row",
                                             name="ones_row0")
            nc.vector.memset(state["ones_row"][:], 1.0)
            state["ident"] = persist.tile([P, P], BF16, tag="ident",
                                          name="ident0")
            nc.scalar.dma_start(state["ident"][:], ins["ident"])
        self.ones_row = state["ones_row"]
        self.ident = state["ident"]
        if rep < 2:
            nc.vector.memset(self.vsb_h[:, :, :, 64:VW], 1.0)

    def xt_load(self, bb):
        xt = self.pools["s1"].tile([P, self.cfg.CC, 512], BF16, tag="xt",
                                   bufs=4, name=f"xt{bb}_{self.rep}")
        self.nc.sync.dma_start(xt[:], self.xT_r[:, bb, :, :])
        return xt

    def qk_quarter(self, bb, m, xt, quarter, st):
        nc = self.nc
        CC = self.cfg.CC
        if quarter == 0:
            st["acc"] = self.pools["ps_acc"].tile(
                [P, 512], F32, tag="acc", name=f"qkacc{bb}_{m}_{self.rep}")
        acc = st["acc"]
        for cc in range(quarter * 2, quarter * 2 + 2):
            nc.tensor.matmul(
                acc[:],
                self.wqk_sb[:, cc, m * P:(m + 1) * P],
                xt[:, cc, :],
                start=(cc == 0),
                stop=(cc == CC - 1),
            )
        if quarter == 3:
            nc.vector.tensor_scalar_add(
                self.qkT[:, m, bb * 512:(bb + 1) * 512], acc[:],
                self.bqk_sb[:, m:m + 1],
            )

    def v_big(self, bb, xt, half, st):
        # v^T [128 v-dims, 512 t] dim-major: 8 matmuls per block instead
        # of 32 token-major ones; transposed below on the PE.
        nc = self.nc
        if half == 0:
            st["vT"] = self.pools["ps_acc"].tile(
                [P, 512], F32, tag="acc", name=f"vT{bb}_{self.rep}")
        for cc in range(half * 4, half * 4 + 4):
            nc.tensor.matmul(
                st["vT"][:],
                self.wv_sb[:, cc, :],
                xt[:, cc, :],
                start=(cc == 0),
                stop=(cc == self.cfg.CC - 1),
            )
        if half == 1:
            vt_sb = self.pools["s2"].tile([P, 512], BF16, tag="vts",
                                          bufs=2, name=f"vts{bb}_{self.rep}")
            nc.scalar.activation(vt_sb[:], st["vT"][:], COPY)
            st["vt_sb"] = vt_sb

    def v_tr(self, bb, ts, st):
        nc = self.nc
        kt_g = bb * 4 + ts
        if ts == 0:
            st["vtr"] = self.pools["ps_acc"].tile(
                [P, 512], BF16, tag="acc", name=f"vtr{bb}_{self.rep}")
        col = slice(ts * P, (ts + 1) * P)
        nc.tensor.transpose(st["vtr"][:, col], st["vt_sb"][:, col],
                            self.ident[:])
        nc.vector.tensor_copy(
            self.vsb_h[:, kt_g, :, 0:64],
            st["vtr"][:, col].rearrange("p (h d) -> p h d", d=D),
        )

    def stage1_fillers(self, bb):
        xt = self.xt_load(bb)
        fs = []
        for m in range(QKCH):
            st = {}
            for q in range(4):
                fs.append(
                    lambda m=m, q=q, st=st: self.qk_quarter(bb, m, xt, q, st))
        vst = {}
        fs.append(lambda st=vst: self.v_big(bb, xt, 0, st))
        fs.append(lambda st=vst: self.v_big(bb, xt, 1, st))
        for ts in range(4):
            fs.append(lambda ts=ts, st=vst: self.v_tr(bb, ts, st))
        return fs


def emit(tc, outs, ins, cfg):
    from contextlib import ExitStack
    with ExitStack() as _stk:
        pools = dict(
            persist=_stk.enter_context(tc.tile_pool(name="persist", bufs=1)),
            s1=_stk.enter_context(tc.tile_pool(name="s1", bufs=2)),
            s2=_stk.enter_context(tc.tile_pool(name="s2", bufs=4)),
            s3=_stk.enter_context(tc.tile_pool(name="s3", bufs=2)),
            dram=_stk.enter_context(
                tc.tile_pool(name="dram", bufs=1, space="DRAM")),
            ps_acc=_stk.enter_context(tc.tile_pool(
                name="ps_acc", bufs=2, space="PSUM")),
            ps_s=_stk.enter_context(tc.tile_pool(
                name="ps_s", bufs=2, space="PSUM")),
            ps_y=_stk.enter_context(tc.tile_pool(
                name="ps_y", bufs=2, space="PSUM")),
        )
        carry = []
        state = {}
        for rep in range(cfg.repeat):
            carry = _emit_once(tc, outs["out"], ins, cfg, rep, pools, carry,
                               state)
        for f in carry:
            f()


def _emit_once(tc, out, ins, cfg, rep, pools, carry, state):
    nc = tc.nc
    CC = cfg.CC
    ctx = _RepCtx(tc, ins, cfg, rep, pools, state)
    for f in ctx.stage1_fillers(0):
        f()

    s2 = pools["s2"]
    s3 = pools["s3"]
    ps_acc = pools["ps_acc"]
    ps_s = pools["ps_s"]
    ps_y = pools["ps_y"]
    qkT, vsb, mask_sb = ctx.qkT, ctx.vsb, ctx.mask_sb
    ones_row = ctx.ones_row

    # ---- proj chunks (for THIS rep; dispensed during the NEXT rep) ----
    a2a_out_r = ctx.a2a_out.rearrange("(c p) t -> p c t", p=P)

    def proj_load(st):
        # full y^T [1024, 512] for my block: 1KB runs per (p, chunk)
        ag = s3.tile([P, CC, 512], BF16, tag="ag")
        nc.gpsimd.dma_start(ag[:], a2a_out_r[:])
        st["ag"] = ag

    def proj_half(tsub, hh, half, st, pst):
        col = slice(tsub * P, (tsub + 1) * P)
        if half == 0:
            pst["op"] = ps_acc.tile([P, 512], F32, tag="acc",
                                    name=f"pacc{tsub}_{hh}_{rep}")
        op = pst["op"]
        for cc in range(half * 4, half * 4 + 4):
            nc.tensor.matmul(
                op[:], st["ag"][:, cc, col],
                ctx.wp_sb[:, cc, hh * 512:(hh + 1) * 512],
                start=(cc == 0), stop=False,
            )
        if half == 1:
            nc.tensor.matmul(
                op[:], ones_row[:1, :],
                ctx.bp_sb[:1, hh * 512:(hh + 1) * 512],
                start=False, stop=True,
            )
            o_sb = s3.tile([P, 512], F32, tag="osb")
            nc.vector.tensor_copy(o_sb[:], op[:])
            nc.sync.dma_start(
                out[tsub * P:(tsub + 1) * P, hh * 512:(hh + 1) * 512],
                o_sb[:])

    def proj_fillers():
        st = {}
        fs = [lambda st=st: proj_load(st)]
        for tsub in range(4):
            for hh in range(2):
                pst = {}
                fs.append(lambda tsub=tsub, hh=hh, pst=pst, st=st:
                          proj_half(tsub, hh, 0, st, pst))
                fs.append(lambda tsub=tsub, hh=hh, pst=pst, st=st:
                          proj_half(tsub, hh, 1, st, pst))
        return fs

    # ---- attention with filler dispensing ----
    def attention(bb, early, late, pre):
        # early: fillers safe to run from the start (stage1 of bb+1)
        # late: fillers needing the previous rep's AllToAll (proj)
        # pre: deferred work (previous block's normalize) that MUST be
        #      emitted before this block's first AV (ps_y WAR).
        b, qb = divmod(bb, 4)
        nkt = 4 * qb + 4
        kt_order = list(range(4 * qb, nkt)) + list(range(0, 4 * qb))
        fillers = list(early)
        late = list(late)
        ci = 0  # chain iterations done

        def dispense(n):
            for _ in range(n):
                if fillers:
                    fillers.pop(0)()

        hs = (0, 1)
        pbs = [slice(h * 64, h * 64 + 64) for h in hs]
        ys = [ps_y.tile([P, 512], F32, tag="y",
                        name=f"y{bb}_{h}_{rep}") for h in hs]
        # iteration list: 4 diagonal singles, then off-diagonal k-tiles
        # in PAIRS — one exp instruction covers both tiles of a head,
        # cutting ACT instruction count and chain sync hops ~30%.
        items = [("d", kt) for kt in range(4 * qb, nkt)]
        items += [("p", kt) for kt in range(0, 4 * qb, 2)]
        n_it = len(items)
        Kbase = b * cfg.T

        def qk_mm(dst, i, kt, lo):
            nc.tensor.matmul(
                dst,
                qkT[pbs[i], 1, Kbase + kt * P:Kbase + (kt + 1) * P],
                qkT[pbs[i], 0, bb * 512 + lo:(bb + 1) * 512],
                start=True, stop=True,
            )

        def av_mm(i, kt, mov, lo, start, stop):
            nc.tensor.matmul(
                ys[i][:, lo:],
                vsb[:, b * cfg.KT + kt, hs[i] * VW:(hs[i] + 1) * VW],
                mov,
                start=start, stop=stop,
            )

        for ki, (kind, kt) in enumerate(items):
            if ki == 0:
                fillers.extend(late)
                late = []
            first, last = ki == 0, ki == n_it - 1
            e = s2.tile([P, 2, 2, 512], BF16, tag="e", bufs=4,
                        name=f"e{bb}_{kt}_{rep}")
            if kind == "d":
                j = kt - 4 * qb
                lo = 128 * j if j > 0 else 0
                s = ps_s.tile([P, 2, 512], F32, tag="s",
                              name=f"s{bb}_{kt}_{rep}")
                for i in range(2):
                    qk_mm(s[:, i, lo:], i, kt, lo)
                # one wide exp + one broadcast mask for both heads
                nc.scalar.activation(
                    e[:, :, 0, lo:], s[:, :, lo:], EXP, scale=0.125)
                nc.vector.tensor_mul(
                    e[:, :, 0, lo:lo + P], e[:, :, 0, lo:lo + P],
                    mask_sb[:, j:j + 1, lo:lo + P].to_broadcast(
                        [P, 2, P]))
            else:
                lo = 0
                for i in range(2):
                    s = ps_s.tile([P, 2, 512], F32, tag="s",
                                  name=f"s{bb}_{kt}_{i}_{rep}")
                    qk_mm(s[:, 0, :], i, kt, 0)
                    qk_mm(s[:, 1, :], i, kt + 1, 0)
                    nc.scalar.activation(
                        e[:, i, :, :], s[:, :, :], EXP, scale=0.125)
            if ki == 0:
                for f in pre:
                    f()
                pre = []
            ci += 1
            # dispense fillers between QK and AV: the filler matmuls
            # hide the exp latency on the in-order PE.
            rem_work = len(fillers) + len(late)
            rem_iter = n_it - ci
            cap = 2 if kind == "d" else 3
            if rem_iter > 0 and rem_work > 0:
                per = (rem_work + rem_iter - 1) // rem_iter
                dispense(min(per, cap))
            for i in range(2):
                av_mm(i, kt, e[:, i, 0, lo:], lo, first, last and kind == "d")
                if kind == "p":
                    av_mm(i, kt + 1, e[:, i, 1, :], 0, False, last)
        for f in pre:
            f()
        fillers.extend(late)
        dispense(len(fillers))
        return [lambda i=i: _normalize(bb, hs[i], ys[i]) for i in (0, 1)]

    def _normalize(bb, h, y):
        # the AV ones-columns replicated the denominator into PSUM
        # partitions 64:128; reciprocal straight into partitions 0:64.
        rec = s2.tile([64, 512], F32R, tag="rec", bufs=3,
                      name=f"rec{bb}_{h}_{rep}")
        with nc.allow_low_precision(
            reason="reciprocal of softmax denominators; ~1e-6"
            " relative is plenty"
        ):
            nc.vector.reciprocal(rec[:], y[64:128, :])
        yn = s2.tile([64, 512], BF16, tag="yn", bufs=3,
                     name=f"yn{bb}_{h}_{rep}")
        nc.vector.tensor_mul(yn[:], y[0:64, :], rec[:])
        # V bias is folded into bp on the host (softmax rows sum to 1,
        # so y_final = yhat + bv and bv@Wp + bp replaces bp).
        nc.sync.dma_start(
            ctx.a2a_in[bb * P + h * D:bb * P + (h + 1) * D, :], yn[:])

    def all_to_all():
        if cfg.fake_collective:
            nc.gpsimd.dma_start(ctx.a2a_out[:], ctx.a2a_in[:])
            return
        nc.gpsimd.collective_compute(
            "AllToAll", mybir.AluOpType.bypass,
            replica_groups=cfg.replica_groups,
            ins=[ctx.a2a_in[:].opt()],
            outs=[ctx.a2a_out[:].opt()],
        )

    # ---- fused pipeline ----
    pre = []
    for bb in range(NB):
        early = ctx.stage1_fillers(bb + 1) if bb + 1 < NB else []
        late = []
        if bb == PROJ_BLOCK - 1 and carry:
            early.append(carry.pop(0))  # the a2a_out load leads by a block
        if bb == PROJ_BLOCK:
            late = list(carry)
            carry = []
        pre = attention(bb, early, late, pre)
    # block 7's normalize must precede the A2A emission
    for f in pre:
        f()
    # leftover carry (repeat=1 case): flush before the collective
    for f in carry:
        f()
    all_to_all()
    # defer this rep's proj (A2A-dependent) into the next rep
    return proj_fillers()


SHAPES = {
    "xT": ((P, NB * (C // P) * 512), BF16),
    "wqk": ((P, (C // P) * QKCH * P), BF16),
    "wv": ((P, (C // P) * HL * D), BF16),
    "bqk": ((P, QKCH), F32),
    "wp": ((P, (C // P) * C), BF16),
    "bp": ((1, C), BF16),
    "masks": ((P, 4, 512), BF16),
    "ident": ((P, P), BF16),
}


def build(cfg, num_devices=None):
    nc = bacc.Bacc("TRN2", target_bir_lowering=False, debug=False,
                   num_devices=num_devices or cfg.n_cores)
    ins = {}
    for name, (shape, dt) in SHAPES.items():
        ins[name] = nc.dram_tensor(
            name, list(shape), dt, kind="ExternalInput").ap()
    outs = {"out": nc.dram_tensor(
        "out", [512, C], F32, kind="ExternalOutput").ap()}
    with tile.TileContext(nc) as tc:
        emit(tc, outs, ins, cfg)
    nc.compile()
    return nc


def make_core_inputs(x_full, c_attn_w, c_attn_b, c_proj_w, c_proj_b, cfg,
                     core):
    import ml_dtypes
    bf = ml_dtypes.bfloat16
    T = cfg.T
    hs = slice(core * HL * D, (core + 1) * HL * D)
    wq = c_attn_w[:, 0 * C:1 * C][:, hs]
    wk = c_attn_w[:, 1 * C:2 * C][:, hs]
    wv = c_attn_w[:, 2 * C:3 * C][:, hs]
    bq = c_attn_b[0 * C:1 * C][hs]
    bk = c_attn_b[1 * C:2 * C][hs]
    bv_full = c_attn_b[2 * C:3 * C]
    bp_eff = c_proj_b + bv_full @ c_proj_w

    pp = np.arange(P)[:, None, None]
    jj = np.arange(4)[None, :, None]
    qq = np.arange(512)[None, None, :]
    masks = (qq >= pp + 128 * jj)

    def chunkp(w):
        # [C, m] -> [P, CC*m]: row p holds chunk-major contiguous runs,
        # so every SBUF load is one long run per partition.
        m = w.shape[1]
        return np.ascontiguousarray(
            w.reshape(C // P, P, m).transpose(1, 0, 2).reshape(P, -1)
            .astype(bf))

    # x^T for BOTH batches, block-major (bb = b*TB + tb):
    # [P, bb, cc, 512]
    xbs = []
    for b in range(B):
        xt = x_full[b, :T].T  # [C, T]
        xbs.append(xt.reshape(C // P, P, T // 512, 512)
                   .transpose(1, 2, 0, 3))  # [P, tb, cc, 512]
    xT2 = np.concatenate(xbs, axis=1).reshape(P, -1)

    return {
        "xT": np.ascontiguousarray(xT2.astype(bf)),
        "wqk": chunkp(np.concatenate([wq, wk], axis=1)),
        "wv": chunkp(wv),
        "bqk": np.ascontiguousarray(
            np.concatenate([bq, bk]).reshape(QKCH, P).T, np.float32),
        "wp": chunkp(c_proj_w),
        "bp": np.ascontiguousarray(bp_eff[None, :].astype(bf)),
        "masks": masks.astype(bf),
        "ident": np.eye(P, dtype=bf),
    }


_CACHE = {}


def kernel(**inputs):
    from concourse.bass_utils import run_bass_kernel_spmd

    cfg = CFG_FULL
    x = np.asarray(inputs["x"], np.float32)
    c_attn_w = np.asarray(inputs["c_attn_w"], np.float32)
    c_attn_b = np.asarray(inputs["c_attn_b"], np.float32)
    c_proj_w = np.asarray(inputs["c_proj_w"], np.float32)
    c_proj_b = np.asarray(inputs["c_proj_b"], np.float32)

    if "nc" not in _CACHE:
        _CACHE["nc"] = build(cfg)
    nc = _CACHE["nc"]
    in_maps = [
        make_core_inputs(x, c_attn_w, c_attn_b, c_proj_w, c_proj_b, cfg,
                         core)
        for core in range(cfg.n_cores)
    ]
    res = run_bass_kernel_spmd(nc, in_maps, core_ids=list(range(cfg.n_cores)))
    out = np.empty((B, T_FULL, C), np.float32)
    for core in range(cfg.n_cores):
        b, tb = divmod(core, cfg.TB)
        out[b, tb * 512:(tb + 1) * 512, :] = res.results[core]["out"]
    return out
